# revision 29
# baseline (speedup 1.0000x reference)
"""Single-level 2D Haar DWT (analysis) on Trainium2, data-parallel over 8 cores.

Input  x       [16, 64, 256, 256] f32
       weights [1, 1] f32 (w = 1/sqrt(2); the transform scales by w^2)
Output (ll [16, 64, 128, 128], highs [16, 64, 3, 128, 128])

Math (per (n, c) plane, 2x2 polyphase a,b,c,d):
    ll = w2*((a+b)+(c+d)); lh = w2*((c+d)-(a+b));
    hl = w2*((b+d)-(a+c)); hh = w2*((d-c)-(b-a))
Computed as: s = w2*(r0 + r1), t = w2*(r1 - r0) over full rows, then
    ll = s_even + s_odd;  lh = t_even + t_odd
    hl = s_odd - s_even;  hh = t_odd - t_even

Sharding: batch dim 16 -> 2 batches per core, fully local (no collectives).
"""

import os

import numpy as np

B, C, H, W = 16, 64, 256, 256
N_CORES = 8
BL = B // N_CORES  # batches per core
GROUP = 4  # planes (channels) per inner tile group
H2, W2 = H // 2, W // 2

# Stash of the last BassKernelResults (for test harness introspection).
last_results = None

# Compiled-program cache keyed by (builder, w2): rebuilding the Bass module
# and re-jitting is expensive; repeated kernel() calls reuse it.
_nc_cache = {}


def _build(w2: float, group: int = GROUP, bufs: int = 4):
    import concourse.bacc as bacc
    import concourse.mybir as mybir
    from concourse.tile import TileContext

    f32 = mybir.dt.float32

    nc = bacc.Bacc()
    x = nc.dram_tensor("x", [BL, C, H, W], f32, kind="ExternalInput")
    ll = nc.dram_tensor("ll", [BL, C, H2, W2], f32, kind="ExternalOutput")
    highs = nc.dram_tensor("highs", [BL, C, 3, H2, W2], f32, kind="ExternalOutput")

    n_groups = BL * C // group

    with TileContext(nc) as tc:
        with tc.tile_pool(name="pool", bufs=bufs) as pool:
            for g in range(n_groups):
                n, c0 = divmod(g * group, C)

                # Load `group` full 256x256 planes; partition p holds rows
                # 2p, 2p+1 of each plane (2 KiB contiguous per plane per
                # partition; the whole transfer is contiguous in HBM).
                xin = pool.tile([128, group * 512], f32, tag="xin")
                xv = xin[:].rearrange("p (j t w) -> p j t w", j=group, t=2)
                nc.sync.dma_start(
                    out=xv,
                    in_=x[n, c0 : c0 + group].rearrange("j (p t) w -> p j t w", t=2),
                )
                # xin *= w2 in place (ACT), so downstream ops are plain adds.
                nc.scalar.mul(xin[:], xin[:], w2)
                r0 = xv[:, :, 0, :]
                r1 = xv[:, :, 1, :]

                s_t = pool.tile([128, group * 256], f32, tag="s_t")
                t_t = pool.tile([128, group * 256], f32, tag="t_t")
                sflat = s_t[:].rearrange("p (j w) -> p j w", j=group)
                tflat = t_t[:].rearrange("p (j w) -> p j w", j=group)

                # s = w2*(r0 + r1), t = w2*(r1 - r0)
                nc.vector.tensor_add(sflat, r0, r1)
                nc.vector.tensor_sub(tflat, r1, r0)

                sv = s_t[:].rearrange("p (j w q) -> p j q w", j=group, q=2)
                tv = t_t[:].rearrange("p (j w q) -> p j q w", j=group, q=2)

                ll_t = pool.tile([128, group * 128], f32, tag="ll_t")
                hi_t = pool.tile([128, group * 384], f32, tag="hi_t")
                llv = ll_t[:].rearrange("p (j w) -> p j w", j=group)
                hiv = hi_t[:].rearrange("p (j k w) -> p j k w", j=group, k=3)

                nc.vector.tensor_add(llv, sv[:, :, 0, :], sv[:, :, 1, :])
                nc.vector.tensor_add(hiv[:, :, 0, :], tv[:, :, 0, :], tv[:, :, 1, :])
                nc.vector.tensor_sub(hiv[:, :, 1, :], sv[:, :, 1, :], sv[:, :, 0, :])
                nc.vector.tensor_sub(hiv[:, :, 2, :], tv[:, :, 1, :], tv[:, :, 0, :])

                nc.scalar.dma_start(
                    out=ll[n, c0 : c0 + group].rearrange("j p w -> p j w"),
                    in_=llv,
                )
                nc.scalar.dma_start(
                    out=highs[n, c0 : c0 + group].rearrange("j k p w -> p j k w"),
                    in_=hiv,
                )
    nc.finalize()  # Bacc.finalize runs compile() (reg alloc, wait splitting)
    return nc


def _build_v2(
    w2: float,
    in_bufs: int = 6,
    out_bufs: int = 3,
    planes_per_set: int = 16,
    chunk_rp: int = 4,
    st_bufs: int | None = 4,
    use_stt: bool = True,
    gp_combines: int = 2,
):
    """Partition = (plane, row-block): planes_per_set planes x (128/pps)
    row-blocks per set.

    Per core: 128 planes -> (128/pps) sets. Each set loads chunks of
    [128, 2048] (each partition: 8 consecutive rows = 8 KiB contiguous),
    computes s/t and the four subbands per chunk, accumulates outputs in two
    big SBUF tiles, then stores ll and highs (8 KiB+ runs) once per set.
    """
    import concourse.bacc as bacc
    import concourse.mybir as mybir
    from concourse.tile import TileContext

    f32 = mybir.dt.float32

    nc = bacc.Bacc()
    x = nc.dram_tensor("x", [BL, C, H, W], f32, kind="ExternalInput")
    ll = nc.dram_tensor("ll", [BL, C, H2, W2], f32, kind="ExternalOutput")
    highs = nc.dram_tensor("highs", [BL, C, 3, H2, W2], f32, kind="ExternalOutput")

    if st_bufs is None:
        st_bufs = in_bufs
    mult = mybir.AluOpType.mult
    addop = mybir.AluOpType.add
    subop = mybir.AluOpType.subtract

    pps = planes_per_set
    nq = 128 // pps  # row-blocks ("quarters") per plane
    rp_per_q = (H // 2) // nq  # row-pairs per block
    n_sets = BL * C // pps
    crp = chunk_rp  # row-pairs per chunk
    chunks = rp_per_q // crp
    orow = rp_per_q  # output rows per partition per band

    with TileContext(nc) as tc:
        with tc.tile_pool(name="pool", bufs=1) as pool:
            for s in range(n_sets):
                n, c0 = divmod(s * pps, C)
                # partition = (c q): pps planes x nq row-blocks
                xq = x[n, c0 : c0 + pps].rearrange(
                    "c (q rp t) w -> c q rp t w", q=nq, t=2
                ).rearrange("c q rp t w -> (c q) rp t w")
                llq = ll[n, c0 : c0 + pps].rearrange(
                    "c (q rp) w -> c q rp w", q=nq
                ).rearrange("c q rp w -> (c q) rp w")
                ll_o = pool.tile([128, orow * 128], f32, tag="ll_o", bufs=out_bufs)
                hi_o = pool.tile(
                    [128, 3 * orow * 128], f32, tag="hi_o", bufs=out_bufs
                )
                llv = ll_o[:].rearrange("p (rp w) -> p rp w", w=128)
                hiv = hi_o[:].rearrange("p (k rp w) -> p k rp w", k=3, w=128)
                for ch in range(chunks):
                    xin = pool.tile([128, crp * 512], f32, tag="xin", bufs=in_bufs)
                    xv = xin[:].rearrange("p (rp t w) -> p rp t w", rp=crp, t=2)
                    nc.sync.dma_start(
                        out=xv, in_=xq[:, crp * ch : crp * ch + crp]
                    )
                    r0 = xv[:, :, 0, :]
                    r1 = xv[:, :, 1, :]

                    s_t = pool.tile([128, crp * 256], f32, tag="s_t", bufs=st_bufs)
                    t_t = pool.tile([128, crp * 256], f32, tag="t_t", bufs=st_bufs)
                    sflat = s_t[:].rearrange("p (rp w) -> p rp w", rp=crp)
                    tflat = t_t[:].rearrange("p (rp w) -> p rp w", rp=crp)
                    if use_stt:
                        # h = w2*r0 (ACT, half the input); s/t fused on DVE:
                        # s = (r1*w2) + h, t = (r1*w2) - h
                        h_t = pool.tile(
                            [128, crp * 256], f32, tag="h_t", bufs=st_bufs
                        )
                        hflat = h_t[:].rearrange("p (rp w) -> p rp w", rp=crp)
                        nc.scalar.mul(hflat, r0, w2)
                        nc.vector.scalar_tensor_tensor(
                            sflat, r1, w2, hflat, op0=mult, op1=addop
                        )
                        nc.vector.scalar_tensor_tensor(
                            tflat, r1, w2, hflat, op0=mult, op1=subop
                        )
                    else:
                        nc.scalar.mul(xin[:], xin[:], w2)
                        nc.vector.tensor_add(sflat, r0, r1)
                        nc.vector.tensor_sub(tflat, r1, r0)

                    sv = s_t[:].rearrange("p (rp w q) -> p rp q w", rp=crp, q=2)
                    tv = t_t[:].rearrange("p (rp w q) -> p rp q w", rp=crp, q=2)
                    lld = llv[:, crp * ch : crp * ch + crp, :]
                    nc.vector.tensor_add(lld, sv[:, :, 0, :], sv[:, :, 1, :])
                    nc.vector.tensor_add(
                        hiv[:, 0, crp * ch : crp * ch + crp, :],
                        tv[:, :, 0, :],
                        tv[:, :, 1, :],
                    )
                    # optionally offload the two subtract combines to GPSIMD
                    eng1 = nc.gpsimd if gp_combines >= 1 else nc.vector
                    eng2 = nc.gpsimd if gp_combines >= 2 else nc.vector
                    eng1.tensor_sub(
                        hiv[:, 1, crp * ch : crp * ch + crp, :],
                        sv[:, :, 1, :],
                        sv[:, :, 0, :],
                    )
                    eng2.tensor_sub(
                        hiv[:, 2, crp * ch : crp * ch + crp, :],
                        tv[:, :, 1, :],
                        tv[:, :, 0, :],
                    )
                nc.scalar.dma_start(out=llq, in_=llv)
                for k in range(3):
                    # 4-dim DRAM AP [c, q, rp, w]; dma_start only requires
                    # matching element count and iteration order.
                    hkq = highs[n, c0 : c0 + pps, k].rearrange(
                        "c (q rp) w -> c q rp w", q=nq
                    )
                    nc.scalar.dma_start(out=hkq, in_=hiv[:, k])
    nc.finalize()
    return nc


def kernel(x, weights):
    global last_results
    from concourse.bass_utils import run_bass_kernel_spmd

    x = np.ascontiguousarray(np.asarray(x, dtype=np.float32))
    wv = np.float32(np.asarray(weights).reshape(-1)[0])
    w2 = float(np.float32(wv * wv))

    builder = _build_v2 if os.environ.get("DWT_V", "2") == "2" else _build
    key = (builder.__name__, w2)
    nc = _nc_cache.get(key)
    if nc is None:
        nc = _nc_cache[key] = builder(w2)
    shards = [
        {"x": np.ascontiguousarray(x[i * BL : (i + 1) * BL])} for i in range(N_CORES)
    ]
    trace = os.environ.get("DWT_TRACE", "0") == "1"
    last_results = run_bass_kernel_spmd(
        nc, shards, core_ids=list(range(N_CORES)), trace=trace
    )
    res = last_results.results
    ll = np.concatenate([r["ll"] for r in res], axis=0)
    highs = np.concatenate([r["highs"] for r in res], axis=0)
    return ll, highs


# revision 30
# speedup vs baseline: 1.2463x; 1.2463x over previous
"""Single-level 2D Haar DWT (analysis) on Trainium2, data-parallel over 8 cores.

Input  x       [16, 64, 256, 256] f32
       weights [1, 1] f32 (w = 1/sqrt(2); the transform scales by w^2)
Output (ll [16, 64, 128, 128], highs [16, 64, 3, 128, 128])

Math (per (n, c) plane, 2x2 polyphase a,b,c,d):
    ll = w2*((a+b)+(c+d)); lh = w2*((c+d)-(a+b));
    hl = w2*((b+d)-(a+c)); hh = w2*((d-c)-(b-a))
Computed as: s = w2*(r0 + r1), t = w2*(r1 - r0) over full rows, then
    ll = s_even + s_odd;  lh = t_even + t_odd
    hl = s_odd - s_even;  hh = t_odd - t_even

Sharding: batch dim 16 -> 2 batches per core, fully local (no collectives).
"""

import os

import numpy as np

B, C, H, W = 16, 64, 256, 256
N_CORES = 8
BL = B // N_CORES  # batches per core
GROUP = 4  # planes (channels) per inner tile group
H2, W2 = H // 2, W // 2

# Stash of the last BassKernelResults (for test harness introspection).
last_results = None

# Compiled-program cache keyed by (builder, w2): rebuilding the Bass module
# and re-jitting is expensive; repeated kernel() calls reuse it.
_nc_cache = {}


def _build(w2: float, group: int = GROUP, bufs: int = 4):
    import concourse.bacc as bacc
    import concourse.mybir as mybir
    from concourse.tile import TileContext

    f32 = mybir.dt.float32

    nc = bacc.Bacc()
    x = nc.dram_tensor("x", [BL, C, H, W], f32, kind="ExternalInput")
    ll = nc.dram_tensor("ll", [BL, C, H2, W2], f32, kind="ExternalOutput")
    highs = nc.dram_tensor("highs", [BL, C, 3, H2, W2], f32, kind="ExternalOutput")

    n_groups = BL * C // group

    with TileContext(nc) as tc:
        with tc.tile_pool(name="pool", bufs=bufs) as pool:
            for g in range(n_groups):
                n, c0 = divmod(g * group, C)

                # Load `group` full 256x256 planes; partition p holds rows
                # 2p, 2p+1 of each plane (2 KiB contiguous per plane per
                # partition; the whole transfer is contiguous in HBM).
                xin = pool.tile([128, group * 512], f32, tag="xin")
                xv = xin[:].rearrange("p (j t w) -> p j t w", j=group, t=2)
                nc.sync.dma_start(
                    out=xv,
                    in_=x[n, c0 : c0 + group].rearrange("j (p t) w -> p j t w", t=2),
                )
                # xin *= w2 in place (ACT), so downstream ops are plain adds.
                nc.scalar.mul(xin[:], xin[:], w2)
                r0 = xv[:, :, 0, :]
                r1 = xv[:, :, 1, :]

                s_t = pool.tile([128, group * 256], f32, tag="s_t")
                t_t = pool.tile([128, group * 256], f32, tag="t_t")
                sflat = s_t[:].rearrange("p (j w) -> p j w", j=group)
                tflat = t_t[:].rearrange("p (j w) -> p j w", j=group)

                # s = w2*(r0 + r1), t = w2*(r1 - r0)
                nc.vector.tensor_add(sflat, r0, r1)
                nc.vector.tensor_sub(tflat, r1, r0)

                sv = s_t[:].rearrange("p (j w q) -> p j q w", j=group, q=2)
                tv = t_t[:].rearrange("p (j w q) -> p j q w", j=group, q=2)

                ll_t = pool.tile([128, group * 128], f32, tag="ll_t")
                hi_t = pool.tile([128, group * 384], f32, tag="hi_t")
                llv = ll_t[:].rearrange("p (j w) -> p j w", j=group)
                hiv = hi_t[:].rearrange("p (j k w) -> p j k w", j=group, k=3)

                nc.vector.tensor_add(llv, sv[:, :, 0, :], sv[:, :, 1, :])
                nc.vector.tensor_add(hiv[:, :, 0, :], tv[:, :, 0, :], tv[:, :, 1, :])
                nc.vector.tensor_sub(hiv[:, :, 1, :], sv[:, :, 1, :], sv[:, :, 0, :])
                nc.vector.tensor_sub(hiv[:, :, 2, :], tv[:, :, 1, :], tv[:, :, 0, :])

                nc.scalar.dma_start(
                    out=ll[n, c0 : c0 + group].rearrange("j p w -> p j w"),
                    in_=llv,
                )
                nc.scalar.dma_start(
                    out=highs[n, c0 : c0 + group].rearrange("j k p w -> p j k w"),
                    in_=hiv,
                )
    nc.finalize()  # Bacc.finalize runs compile() (reg alloc, wait splitting)
    return nc


def _build_v2(
    w2: float,
    in_bufs: int = 6,
    out_bufs: int = 3,
    planes_per_set: int = 16,
    chunk_rp: int = 4,
    st_bufs: int | None = 4,
    use_stt: bool = True,
    gp_combines: int = 0,
):
    """Partition = (plane, row-block): planes_per_set planes x (128/pps)
    row-blocks per set.

    Per core: 128 planes -> (128/pps) sets. Each set loads chunks of
    [128, 2048] (each partition: 8 consecutive rows = 8 KiB contiguous),
    computes s/t and the four subbands per chunk, accumulates outputs in two
    big SBUF tiles, then stores ll and highs (8 KiB+ runs) once per set.
    """
    import concourse.bacc as bacc
    import concourse.mybir as mybir
    from concourse.tile import TileContext

    f32 = mybir.dt.float32

    nc = bacc.Bacc()
    x = nc.dram_tensor("x", [BL, C, H, W], f32, kind="ExternalInput")
    ll = nc.dram_tensor("ll", [BL, C, H2, W2], f32, kind="ExternalOutput")
    highs = nc.dram_tensor("highs", [BL, C, 3, H2, W2], f32, kind="ExternalOutput")

    if st_bufs is None:
        st_bufs = in_bufs
    mult = mybir.AluOpType.mult
    addop = mybir.AluOpType.add
    subop = mybir.AluOpType.subtract

    pps = planes_per_set
    nq = 128 // pps  # row-blocks ("quarters") per plane
    rp_per_q = (H // 2) // nq  # row-pairs per block
    n_sets = BL * C // pps
    crp = chunk_rp  # row-pairs per chunk
    chunks = rp_per_q // crp
    orow = rp_per_q  # output rows per partition per band

    with TileContext(nc) as tc:
        with tc.tile_pool(name="pool", bufs=1) as pool:
            for s in range(n_sets):
                n, c0 = divmod(s * pps, C)
                # partition = (c q): pps planes x nq row-blocks
                xq = x[n, c0 : c0 + pps].rearrange(
                    "c (q rp t) w -> c q rp t w", q=nq, t=2
                ).rearrange("c q rp t w -> (c q) rp t w")
                llq = ll[n, c0 : c0 + pps].rearrange(
                    "c (q rp) w -> c q rp w", q=nq
                ).rearrange("c q rp w -> (c q) rp w")
                ll_o = pool.tile([128, orow * 128], f32, tag="ll_o", bufs=out_bufs)
                hi_o = pool.tile(
                    [128, 3 * orow * 128], f32, tag="hi_o", bufs=out_bufs
                )
                llv = ll_o[:].rearrange("p (rp w) -> p rp w", w=128)
                hiv = hi_o[:].rearrange("p (k rp w) -> p k rp w", k=3, w=128)
                for ch in range(chunks):
                    xin = pool.tile([128, crp * 512], f32, tag="xin", bufs=in_bufs)
                    xv = xin[:].rearrange("p (rp t w) -> p rp t w", rp=crp, t=2)
                    nc.sync.dma_start(
                        out=xv, in_=xq[:, crp * ch : crp * ch + crp]
                    )
                    r0 = xv[:, :, 0, :]
                    r1 = xv[:, :, 1, :]

                    s_t = pool.tile([128, crp * 256], f32, tag="s_t", bufs=st_bufs)
                    t_t = pool.tile([128, crp * 256], f32, tag="t_t", bufs=st_bufs)
                    sflat = s_t[:].rearrange("p (rp w) -> p rp w", rp=crp)
                    tflat = t_t[:].rearrange("p (rp w) -> p rp w", rp=crp)
                    if use_stt:
                        # h = w2*r0 (ACT, half the input); s/t fused on DVE:
                        # s = (r1*w2) + h, t = (r1*w2) - h
                        h_t = pool.tile(
                            [128, crp * 256], f32, tag="h_t", bufs=st_bufs
                        )
                        hflat = h_t[:].rearrange("p (rp w) -> p rp w", rp=crp)
                        nc.scalar.mul(hflat, r0, w2)
                        nc.vector.scalar_tensor_tensor(
                            sflat, r1, w2, hflat, op0=mult, op1=addop
                        )
                        nc.vector.scalar_tensor_tensor(
                            tflat, r1, w2, hflat, op0=mult, op1=subop
                        )
                    else:
                        nc.scalar.mul(xin[:], xin[:], w2)
                        nc.vector.tensor_add(sflat, r0, r1)
                        nc.vector.tensor_sub(tflat, r1, r0)

                    sv = s_t[:].rearrange("p (rp w q) -> p rp q w", rp=crp, q=2)
                    tv = t_t[:].rearrange("p (rp w q) -> p rp q w", rp=crp, q=2)
                    lld = llv[:, crp * ch : crp * ch + crp, :]
                    nc.vector.tensor_add(lld, sv[:, :, 0, :], sv[:, :, 1, :])
                    nc.vector.tensor_add(
                        hiv[:, 0, crp * ch : crp * ch + crp, :],
                        tv[:, :, 0, :],
                        tv[:, :, 1, :],
                    )
                    # optionally offload the two subtract combines to GPSIMD
                    eng1 = nc.gpsimd if gp_combines >= 1 else nc.vector
                    eng2 = nc.gpsimd if gp_combines >= 2 else nc.vector
                    eng1.tensor_sub(
                        hiv[:, 1, crp * ch : crp * ch + crp, :],
                        sv[:, :, 1, :],
                        sv[:, :, 0, :],
                    )
                    eng2.tensor_sub(
                        hiv[:, 2, crp * ch : crp * ch + crp, :],
                        tv[:, :, 1, :],
                        tv[:, :, 0, :],
                    )
                nc.scalar.dma_start(out=llq, in_=llv)
                for k in range(3):
                    # 4-dim DRAM AP [c, q, rp, w]; dma_start only requires
                    # matching element count and iteration order.
                    hkq = highs[n, c0 : c0 + pps, k].rearrange(
                        "c (q rp) w -> c q rp w", q=nq
                    )
                    nc.scalar.dma_start(out=hkq, in_=hiv[:, k])
    nc.finalize()
    return nc


def kernel(x, weights):
    global last_results
    from concourse.bass_utils import run_bass_kernel_spmd

    x = np.ascontiguousarray(np.asarray(x, dtype=np.float32))
    wv = np.float32(np.asarray(weights).reshape(-1)[0])
    w2 = float(np.float32(wv * wv))

    builder = _build_v2 if os.environ.get("DWT_V", "2") == "2" else _build
    key = (builder.__name__, w2)
    nc = _nc_cache.get(key)
    if nc is None:
        nc = _nc_cache[key] = builder(w2)
    shards = [
        {"x": np.ascontiguousarray(x[i * BL : (i + 1) * BL])} for i in range(N_CORES)
    ]
    trace = os.environ.get("DWT_TRACE", "0") == "1"
    last_results = run_bass_kernel_spmd(
        nc, shards, core_ids=list(range(N_CORES)), trace=trace
    )
    res = last_results.results
    ll = np.concatenate([r["ll"] for r in res], axis=0)
    highs = np.concatenate([r["highs"] for r in res], axis=0)
    return ll, highs


# revision 31
# speedup vs baseline: 2.4931x; 2.0005x over previous
"""Single-level 2D Haar DWT (analysis) on Trainium2, data-parallel over 8 cores.

Input  x       [16, 64, 256, 256] f32
       weights [1, 1] f32 (w = 1/sqrt(2); the transform scales by w^2)
Output (ll [16, 64, 128, 128], highs [16, 64, 3, 128, 128])

Math (per (n, c) plane, 2x2 polyphase a,b,c,d):
    ll = w2*((a+b)+(c+d)); lh = w2*((c+d)-(a+b));
    hl = w2*((b+d)-(a+c)); hh = w2*((d-c)-(b-a))
Computed as: s = w2*(r0 + r1), t = w2*(r1 - r0) over full rows, then
    ll = s_even + s_odd;  lh = t_even + t_odd
    hl = s_odd - s_even;  hh = t_odd - t_even

Sharding: batch dim 16 -> 2 batches per core, fully local (no collectives).
"""

import os

import numpy as np

B, C, H, W = 16, 64, 256, 256
N_CORES = 8
BL = B // N_CORES  # batches per core
GROUP = 4  # planes (channels) per inner tile group
H2, W2 = H // 2, W // 2

# Stash of the last BassKernelResults (for test harness introspection).
last_results = None

# Compiled-program cache keyed by (builder, w2): rebuilding the Bass module
# and re-jitting is expensive; repeated kernel() calls reuse it.
_nc_cache = {}


def _build(w2: float, group: int = GROUP, bufs: int = 4):
    import concourse.bacc as bacc
    import concourse.mybir as mybir
    from concourse.tile import TileContext

    f32 = mybir.dt.float32

    nc = bacc.Bacc()
    x = nc.dram_tensor("x", [BL, C, H, W], f32, kind="ExternalInput")
    ll = nc.dram_tensor("ll", [BL, C, H2, W2], f32, kind="ExternalOutput")
    highs = nc.dram_tensor("highs", [BL, C, 3, H2, W2], f32, kind="ExternalOutput")

    n_groups = BL * C // group

    with TileContext(nc) as tc:
        with tc.tile_pool(name="pool", bufs=bufs) as pool:
            for g in range(n_groups):
                n, c0 = divmod(g * group, C)

                # Load `group` full 256x256 planes; partition p holds rows
                # 2p, 2p+1 of each plane (2 KiB contiguous per plane per
                # partition; the whole transfer is contiguous in HBM).
                xin = pool.tile([128, group * 512], f32, tag="xin")
                xv = xin[:].rearrange("p (j t w) -> p j t w", j=group, t=2)
                nc.sync.dma_start(
                    out=xv,
                    in_=x[n, c0 : c0 + group].rearrange("j (p t) w -> p j t w", t=2),
                )
                # xin *= w2 in place (ACT), so downstream ops are plain adds.
                nc.scalar.mul(xin[:], xin[:], w2)
                r0 = xv[:, :, 0, :]
                r1 = xv[:, :, 1, :]

                s_t = pool.tile([128, group * 256], f32, tag="s_t")
                t_t = pool.tile([128, group * 256], f32, tag="t_t")
                sflat = s_t[:].rearrange("p (j w) -> p j w", j=group)
                tflat = t_t[:].rearrange("p (j w) -> p j w", j=group)

                # s = w2*(r0 + r1), t = w2*(r1 - r0)
                nc.vector.tensor_add(sflat, r0, r1)
                nc.vector.tensor_sub(tflat, r1, r0)

                sv = s_t[:].rearrange("p (j w q) -> p j q w", j=group, q=2)
                tv = t_t[:].rearrange("p (j w q) -> p j q w", j=group, q=2)

                ll_t = pool.tile([128, group * 128], f32, tag="ll_t")
                hi_t = pool.tile([128, group * 384], f32, tag="hi_t")
                llv = ll_t[:].rearrange("p (j w) -> p j w", j=group)
                hiv = hi_t[:].rearrange("p (j k w) -> p j k w", j=group, k=3)

                nc.vector.tensor_add(llv, sv[:, :, 0, :], sv[:, :, 1, :])
                nc.vector.tensor_add(hiv[:, :, 0, :], tv[:, :, 0, :], tv[:, :, 1, :])
                nc.vector.tensor_sub(hiv[:, :, 1, :], sv[:, :, 1, :], sv[:, :, 0, :])
                nc.vector.tensor_sub(hiv[:, :, 2, :], tv[:, :, 1, :], tv[:, :, 0, :])

                nc.scalar.dma_start(
                    out=ll[n, c0 : c0 + group].rearrange("j p w -> p j w"),
                    in_=llv,
                )
                nc.scalar.dma_start(
                    out=highs[n, c0 : c0 + group].rearrange("j k p w -> p j k w"),
                    in_=hiv,
                )
    nc.finalize()  # Bacc.finalize runs compile() (reg alloc, wait splitting)
    return nc


def _build_v2(
    w2: float,
    in_bufs: int = 8,
    out_bufs: int = 3,
    planes_per_set: int = 16,
    chunk_rp: int = 4,
    st_bufs: int | None = 3,
    use_stt: bool = True,
    gp_combines: int = 0,
):
    """Partition = (plane, row-block): planes_per_set planes x (128/pps)
    row-blocks per set.

    Per core: 128 planes -> (128/pps) sets. Each set loads chunks of
    [128, 2048] (each partition: 8 consecutive rows = 8 KiB contiguous),
    computes s/t and the four subbands per chunk, accumulates outputs in two
    big SBUF tiles, then stores ll and highs (8 KiB+ runs) once per set.
    """
    import concourse.bacc as bacc
    import concourse.mybir as mybir
    from concourse.tile import TileContext

    f32 = mybir.dt.float32

    nc = bacc.Bacc()
    x = nc.dram_tensor("x", [BL, C, H, W], f32, kind="ExternalInput")
    ll = nc.dram_tensor("ll", [BL, C, H2, W2], f32, kind="ExternalOutput")
    highs = nc.dram_tensor("highs", [BL, C, 3, H2, W2], f32, kind="ExternalOutput")

    if st_bufs is None:
        st_bufs = in_bufs
    mult = mybir.AluOpType.mult
    addop = mybir.AluOpType.add
    subop = mybir.AluOpType.subtract

    pps = planes_per_set
    nq = 128 // pps  # row-blocks ("quarters") per plane
    rp_per_q = (H // 2) // nq  # row-pairs per block
    n_sets = BL * C // pps
    crp = chunk_rp  # row-pairs per chunk
    chunks = rp_per_q // crp
    orow = rp_per_q  # output rows per partition per band

    with TileContext(nc) as tc:
        with tc.tile_pool(name="pool", bufs=1) as pool:
            for s in range(n_sets):
                n, c0 = divmod(s * pps, C)
                # partition = (c q): pps planes x nq row-blocks
                xq = x[n, c0 : c0 + pps].rearrange(
                    "c (q rp t) w -> c q rp t w", q=nq, t=2
                ).rearrange("c q rp t w -> (c q) rp t w")
                llq = ll[n, c0 : c0 + pps].rearrange(
                    "c (q rp) w -> c q rp w", q=nq
                ).rearrange("c q rp w -> (c q) rp w")
                ll_o = pool.tile([128, orow * 128], f32, tag="ll_o", bufs=out_bufs)
                hi_o = pool.tile(
                    [128, 3 * orow * 128], f32, tag="hi_o", bufs=out_bufs
                )
                llv = ll_o[:].rearrange("p (rp w) -> p rp w", w=128)
                hiv = hi_o[:].rearrange("p (k rp w) -> p k rp w", k=3, w=128)
                for ch in range(chunks):
                    xin = pool.tile([128, crp * 512], f32, tag="xin", bufs=in_bufs)
                    xv = xin[:].rearrange("p (rp t w) -> p rp t w", rp=crp, t=2)
                    nc.sync.dma_start(
                        out=xv, in_=xq[:, crp * ch : crp * ch + crp]
                    )
                    r0 = xv[:, :, 0, :]
                    r1 = xv[:, :, 1, :]

                    s_t = pool.tile([128, crp * 256], f32, tag="s_t", bufs=st_bufs)
                    t_t = pool.tile([128, crp * 256], f32, tag="t_t", bufs=st_bufs)
                    sflat = s_t[:].rearrange("p (rp w) -> p rp w", rp=crp)
                    tflat = t_t[:].rearrange("p (rp w) -> p rp w", rp=crp)
                    if use_stt:
                        # h = w2*r0 (ACT, half the input); s/t fused on DVE:
                        # s = (r1*w2) + h, t = (r1*w2) - h
                        h_t = pool.tile(
                            [128, crp * 256], f32, tag="h_t", bufs=st_bufs
                        )
                        hflat = h_t[:].rearrange("p (rp w) -> p rp w", rp=crp)
                        nc.scalar.mul(hflat, r0, w2)
                        nc.vector.scalar_tensor_tensor(
                            sflat, r1, w2, hflat, op0=mult, op1=addop
                        )
                        nc.vector.scalar_tensor_tensor(
                            tflat, r1, w2, hflat, op0=mult, op1=subop
                        )
                    else:
                        nc.scalar.mul(xin[:], xin[:], w2)
                        nc.vector.tensor_add(sflat, r0, r1)
                        nc.vector.tensor_sub(tflat, r1, r0)

                    sv = s_t[:].rearrange("p (rp w q) -> p rp q w", rp=crp, q=2)
                    tv = t_t[:].rearrange("p (rp w q) -> p rp q w", rp=crp, q=2)
                    lld = llv[:, crp * ch : crp * ch + crp, :]
                    nc.vector.tensor_add(lld, sv[:, :, 0, :], sv[:, :, 1, :])
                    nc.vector.tensor_add(
                        hiv[:, 0, crp * ch : crp * ch + crp, :],
                        tv[:, :, 0, :],
                        tv[:, :, 1, :],
                    )
                    # optionally offload the two subtract combines to GPSIMD
                    eng1 = nc.gpsimd if gp_combines >= 1 else nc.vector
                    eng2 = nc.gpsimd if gp_combines >= 2 else nc.vector
                    eng1.tensor_sub(
                        hiv[:, 1, crp * ch : crp * ch + crp, :],
                        sv[:, :, 1, :],
                        sv[:, :, 0, :],
                    )
                    eng2.tensor_sub(
                        hiv[:, 2, crp * ch : crp * ch + crp, :],
                        tv[:, :, 1, :],
                        tv[:, :, 0, :],
                    )
                nc.scalar.dma_start(out=llq, in_=llv)
                for k in range(3):
                    # 4-dim DRAM AP [c, q, rp, w]; dma_start only requires
                    # matching element count and iteration order.
                    hkq = highs[n, c0 : c0 + pps, k].rearrange(
                        "c (q rp) w -> c q rp w", q=nq
                    )
                    nc.scalar.dma_start(out=hkq, in_=hiv[:, k])
    nc.finalize()
    return nc


def kernel(x, weights):
    global last_results
    from concourse.bass_utils import run_bass_kernel_spmd

    x = np.ascontiguousarray(np.asarray(x, dtype=np.float32))
    wv = np.float32(np.asarray(weights).reshape(-1)[0])
    w2 = float(np.float32(wv * wv))

    builder = _build_v2 if os.environ.get("DWT_V", "2") == "2" else _build
    key = (builder.__name__, w2)
    nc = _nc_cache.get(key)
    if nc is None:
        nc = _nc_cache[key] = builder(w2)
    shards = [
        {"x": np.ascontiguousarray(x[i * BL : (i + 1) * BL])} for i in range(N_CORES)
    ]
    trace = os.environ.get("DWT_TRACE", "0") == "1"
    last_results = run_bass_kernel_spmd(
        nc, shards, core_ids=list(range(N_CORES)), trace=trace
    )
    res = last_results.results
    ll = np.concatenate([r["ll"] for r in res], axis=0)
    highs = np.concatenate([r["highs"] for r in res], axis=0)
    return ll, highs
